# revision 3
# baseline (speedup 1.0000x reference)
"""PrRoIPool2D (precise ROI pooling) Trainium2 kernel — 8-core SPMD.

Strategy ("fused banded sweep"):
  out[r,c,p,q] = sum_{h,w} F[b_r,c,h,w] * Iy[r,p,h] * Ix[r,q,w]
The (Iy ⊗ Ix) basis is banded: bin (r,p) touches only a ~5-row window of h.
Host packs, per core (= one feature batch), a basis tensor B whose columns are
(r,p,q) output columns sorted by h-window start; for each 2-row h-chunk k the
alive columns form one contiguous interval [LO_k, HI_k).  The device then runs
one matmul per (chunk, c-half, psum-bank-piece) with the features as stationary
weights, PSUM-accumulating straight into the final output columns (per-element
has_written semantics make first-write overwrite, later writes add).  No
intermediate tensor is ever evacuated — only the final [256, COLS] output.

Everything F-dependent runs on device; the host only does O(R*(H+W)) coordinate
preprocessing (tent-basis integrals, sorting, packing) and output unpermutation.
"""

import numpy as np
import ml_dtypes

POOLED = 7
SCALE = 0.5
N, C, H, W = 8, 256, 56, 56
NCORES = 8
CHUNK_H = 2
NCHUNK = H // CHUNK_H          # 28
KDIM = CHUNK_H * W             # 112 (payload rows)
KPAD = 128                     # device K rows (padded for fast weight load)
SIM_SAFE = False               # True: split MMs for CoreSim's uniformity assert
BANK = 512                     # fp32 elements per PSUM bank
BF16 = ml_dtypes.bfloat16

_kernel_cache = {}
LAST_RESULTS = None            # BassKernelResults stash for test harnesses


def _tent_integral(start, end, n):
    i = np.arange(n, dtype=np.float64)
    a = np.clip(start[..., None] - i, -1.0, 1.0)
    b = np.clip(end[..., None] - i, -1.0, 1.0)

    def G(t):
        return np.where(t <= 0.0, 0.5 * (t + 1.0) ** 2, 1.0 - 0.5 * (1.0 - t) ** 2)

    return G(b) - G(a)


def _host_prep(features, rois):
    """Build per-core packed device inputs + unpack metadata."""
    R = rois.shape[0]
    batch = rois[:, 0].astype(np.int32)
    x1 = rois[:, 1].astype(np.float64) * SCALE
    y1 = rois[:, 2].astype(np.float64) * SCALE
    x2 = rois[:, 3].astype(np.float64) * SCALE
    y2 = rois[:, 4].astype(np.float64) * SCALE
    bw = (x2 - x1) / POOLED
    bh = (y2 - y1) / POOLED
    pw = np.arange(POOLED, dtype=np.float64)
    xs = x1[:, None] + pw * bw[:, None]
    ys = y1[:, None] + pw * bh[:, None]
    Ix = _tent_integral(xs, xs + bw[:, None], W)       # [R,7,W]
    Iy = _tent_integral(ys, ys + bh[:, None], H)       # [R,7,H]
    area = bw * bh
    scl = np.where(area > 0, 1.0 / np.maximum(area, 1e-12), 0.0)
    Iy_s = Iy * scl[:, None, None]

    core_rois = [np.nonzero(batch == c)[0] for c in range(NCORES)]
    Rmax = max(len(ix) for ix in core_rois)
    NGRP = Rmax * POOLED
    COLS = NGRP * POOLED
    NBANK = (COLS + BANK - 1) // BANK
    COLS_PAD = NBANK * BANK

    # per-core sorted group windows
    meta = []
    for c in range(NCORES):
        idx = core_rois[c]
        wins = []
        for rg in idx:
            for p in range(POOLED):
                nz = np.nonzero(Iy_s[rg, p] != 0)[0]
                lo, hi = (int(nz[0]), int(nz[-1])) if len(nz) else (0, 0)
                wins.append((lo, hi, rg, p))
        nd = (Rmax - len(idx)) * POOLED
        for dnum in range(nd):
            hf = (dnum * H) // max(nd, 1)
            wins.append((hf, hf, -1, -1))
        wins.sort(key=lambda t: (t[0], t[1]))
        meta.append(wins)

    # per-chunk alive interval (union over cores), in group units
    LO = np.full(NCHUNK, NGRP, dtype=np.int64)
    HI = np.zeros(NCHUNK, dtype=np.int64)
    for c in range(NCORES):
        wins = meta[c]
        lo_arr = np.array([w[0] for w in wins])
        hi_arr = np.array([w[1] for w in wins])
        for k in range(NCHUNK):
            h0, h1 = CHUNK_H * k, CHUNK_H * k + CHUNK_H - 1
            alive = np.nonzero((lo_arr <= h1) & (hi_arr >= h0))[0]
            if len(alive):
                LO[k] = min(LO[k], alive[0])
                HI[k] = max(HI[k], alive[-1] + 1)
    active = HI > 0
    LOc, HIc = LO * POOLED, HI * POOLED

    offs = np.zeros(NCHUNK + 1, dtype=np.int64)
    for k in range(NCHUNK):
        offs[k + 1] = offs[k] + (int(HIc[k] - LOc[k]) if active[k] else 0)
    NB = int(offs[-1])

    # pack B (bf16) per core: B[(dh,w), packed_col]
    B = np.zeros((NCORES, KDIM, NB), dtype=np.float32)
    IxT = Ix.transpose(0, 2, 1)                        # [R, W, 7]
    for c in range(NCORES):
        wins = meta[c]
        for k in range(NCHUNK):
            if not active[k]:
                continue
            for g in range(int(LO[k]), int(HI[k])):
                wlo, whi, rg, p = wins[g]
                if rg < 0:
                    continue
                cb = int(offs[k]) + (g * POOLED - int(LOc[k]))
                for dh in range(CHUNK_H):
                    h = CHUNK_H * k + dh
                    if wlo <= h <= whi:
                        B[c, dh * W:(dh + 1) * W, cb:cb + POOLED] = (
                            Iy_s[rg, p, h] * IxT[rg]
                        )
    B = np.pad(B, ((0, 0), (0, KPAD - KDIM), (0, 0))).astype(BF16)

    # features per core, chunk-major transposed: FT[(dh,w), k*C + cc]
    f = features.astype(np.float32)                    # [N,C,H,W]
    # [N, C, k, dh, w] -> [N, dh, w, k, C]
    ft = f.reshape(N, C, NCHUNK, CHUNK_H, W).transpose(0, 3, 4, 2, 1)
    FT = np.pad(ft.reshape(N, KDIM, NCHUNK * C),
                ((0, 0), (0, KPAD - KDIM), (0, 0))).astype(BF16)

    return dict(B=B, FT=FT, offs=offs, LOc=LOc.astype(int), HIc=HIc.astype(int),
                active=active, meta=meta, Rmax=Rmax, COLS=COLS,
                COLS_PAD=COLS_PAD, NBANK=NBANK, NB=NB, R=R)


def shape_cols(LOc, HIc, active):
    return max(int(HIc[k]) for k in range(NCHUNK) if active[k])


def _build_bass(shape_key):
    """Build + compile the SPMD Bass program for given packing metadata."""
    NB, COLS_PAD, NBANK, LOc, HIc, active_t, offs = shape_key
    LOc, HIc, active, offs = list(LOc), list(HIc), list(active_t), list(offs)

    import concourse.bass as bass  # noqa: F401
    import concourse.tile as tile
    from concourse import bacc, mybir

    nc = bacc.Bacc("TRN2", target_bir_lowering=False, debug=False,
                   enable_asserts=False, num_devices=NCORES)
    bf = mybir.dt.bfloat16
    f32 = mybir.dt.float32
    ft_ap = nc.dram_tensor("ft", [KPAD, NCHUNK * C], bf, kind="ExternalInput").ap()
    b_ap = nc.dram_tensor("bb", [KPAD, NB], bf, kind="ExternalInput").ap()
    COLS = shape_cols(LOc, HIc, active)
    out_ap = nc.dram_tensor("out", [C, COLS], f32, kind="ExternalOutput").ap()

    kact = [k for k in range(NCHUNK) if active[k]]
    # last chunk touching each bank (per-bank stop flag)
    last_k = {}
    for k in kact:
        for bk in range(LOc[k] // BANK, (HIc[k] - 1) // BANK + 1):
            last_k[bk] = k

    with tile.TileContext(nc) as tc:
        with (
            tc.tile_pool(name="ftp", bufs=1) as ftp,
            tc.tile_pool(name="bp", bufs=1) as bp,
            tc.tile_pool(name="pp", bufs=8, space="PSUM") as pp,
            tc.tile_pool(name="op", bufs=2) as op,
        ):
            ft_sb = ftp.tile([KPAD, NCHUNK * C], bf)
            b_sb = bp.tile([KPAD, NB], bf)
            # split input DMAs so early chunks' matmuls can start sooner
            NSPLIT = 4
            for s in range(NSPLIT):
                k0, k1 = (NCHUNK * s) // NSPLIT, (NCHUNK * (s + 1)) // NSPLIT
                nc.sync.dma_start(ft_sb[:, k0 * C:k1 * C], ft_ap[:, k0 * C:k1 * C])
                o0, o1 = offs[k0], offs[k1]
                if o1 > o0:
                    nc.scalar.dma_start(b_sb[:, o0:o1], b_ap[:, o0:o1])

            for m in range(2):
                ptiles = [pp.tile([128, BANK], f32, tag="bank", name=f"pt{m}_{i}") for i in range(NBANK)]
                # cols written so far per bank (has_written high-water mark);
                # -1 = bank untouched.  Intervals are monotone, so each new
                # matmul piece splits into an all-accumulate part (< mark) and
                # an all-fresh part (>= mark) — keeps sim's uniformity assert
                # happy and matches per-element HW semantics.
                whi = [-1] * NBANK
                for k in kact:
                    lo, hi, ob = LOc[k], HIc[k], offs[k]
                    lhsT = ft_sb[:, k * C + m * 128: k * C + (m + 1) * 128]
                    for bk in range(lo // BANK, (hi - 1) // BANK + 1):
                        s = max(lo, bk * BANK)
                        e = min(hi, (bk + 1) * BANK)
                        is_last = k == last_k[bk]
                        if whi[bk] < 0:
                            pieces = [(s, e, True)]
                        elif SIM_SAFE:
                            pieces = []
                            if s < whi[bk]:
                                pieces.append((s, min(e, whi[bk]), False))
                            if e > whi[bk]:
                                pieces.append((max(s, whi[bk]), e, False))
                        else:
                            pieces = [(s, e, False)]
                        for pi, (ps, pe, st) in enumerate(pieces):
                            nc.tensor.matmul(
                                ptiles[bk][:, ps - bk * BANK: pe - bk * BANK],
                                lhsT=lhsT,
                                rhs=b_sb[:, ob + ps - lo: ob + pe - lo],
                                start=st,
                                stop=is_last and pi == len(pieces) - 1,
                            )
                        whi[bk] = max(whi[bk], e)
                out_sb = op.tile([128, COLS], f32)
                for bk in range(NBANK):
                    w = min(BANK, COLS - bk * BANK)
                    dst = out_sb[:, bk * BANK: bk * BANK + w]
                    if bk % 2 == 0:
                        nc.vector.tensor_copy(dst, ptiles[bk][:, :w])
                    else:
                        nc.scalar.copy(dst, ptiles[bk][:, :w])
                    nc.sync.dma_start(
                        out_ap[m * 128:(m + 1) * 128, bk * BANK: bk * BANK + w],
                        dst)

    nc.compile()
    return nc


def kernel(features, rois):
    global LAST_RESULTS
    import os
    from concourse import bass_utils

    features = np.asarray(features, dtype=np.float32)
    rois = np.asarray(rois, dtype=np.float32)
    hp = _host_prep(features, rois)

    shape_key = (hp["NB"], hp["COLS_PAD"], hp["NBANK"],
                 tuple(hp["LOc"]), tuple(hp["HIc"]),
                 tuple(bool(a) for a in hp["active"]),
                 tuple(int(o) for o in hp["offs"]))
    nc = _kernel_cache.get(shape_key)
    if nc is None:
        nc = _build_bass(shape_key)
        _kernel_cache[shape_key] = nc

    in_maps = [{"ft": np.ascontiguousarray(hp["FT"][c]),
                "bb": np.ascontiguousarray(hp["B"][c])}
               for c in range(NCORES)]
    res = bass_utils.run_bass_kernel_spmd(nc, in_maps, core_ids=list(range(NCORES)),
                                          tmpdir=os.environ.get("BASS_TMPDIR"))
    LAST_RESULTS = res

    # unpack: out_core[c_chan, col(g,q)] -> final[r, c_chan, p, q]
    final = np.zeros((hp["R"], C, POOLED, POOLED), dtype=np.float32)
    for c in range(NCORES):
        out = res.results[c]["out"]                    # [C, COLS]
        wins = hp["meta"][c]
        gidx = [g for g, (_, _, rg, _) in enumerate(wins) if rg >= 0]
        if not gidx:
            continue
        rgs = np.array([wins[g][2] for g in gidx])
        ps = np.array([wins[g][3] for g in gidx])
        cols = out.reshape(C, -1, POOLED)[:, gidx, :]  # [C, ngrp, 7]
        final[rgs, :, ps, :] = cols.transpose(1, 0, 2)
    return final



# revision 6
# speedup vs baseline: 1.0455x; 1.0455x over previous
"""PrRoIPool2D (precise ROI pooling) Trainium2 kernel — 8-core SPMD.

Fused banded sweep with aligned slot packing:
  out[r,c,p,q] = sum_{h,w} F[b_r,c,h,w] * Iy[r,p,h] * Ix[r,q,w]
Each (r,p) "group" is alive over a ~2-3 h-chunk window.  Groups are assigned
to output SLOTS via per-entry-chunk quotas (quota[k] = max over cores of
groups entering at chunk k), which aligns every core's alive window to the
same slot interval — the SPMD union of per-chunk matmul intervals is then
tight.  Slots partition into PSUM banks (blocks); a zero-weights matmul
clears each bank's has_written before use (doubling as PE clock warm-up),
then per-chunk matmuls accumulate F-chunk x B-columns straight into the
final output columns.  Output is evacuated per bank as bf16.
"""

import numpy as np
import ml_dtypes

POOLED = 7
SCALE = 0.5
N, C, H, W = 8, 256, 56, 56
NCORES = 8
CHUNK_H = 2
NCHUNK = H // CHUNK_H          # 28
KDIM = CHUNK_H * W             # 112 device K rows (no padding)
BANK = 512                     # fp32 elements per PSUM bank
MAXSLOT_PER_BLOCK = BANK // POOLED  # 73
BF16 = ml_dtypes.bfloat16

_kernel_cache = {}
LAST_RESULTS = None            # BassKernelResults stash for test harnesses


def _tent_integral(start, end, n):
    i = np.arange(n, dtype=np.float64)
    a = np.clip(start[..., None] - i, -1.0, 1.0)
    b = np.clip(end[..., None] - i, -1.0, 1.0)

    def G(t):
        return np.where(t <= 0.0, 0.5 * (t + 1.0) ** 2, 1.0 - 0.5 * (1.0 - t) ** 2)

    return G(b) - G(a)


def _host_prep(features, rois):
    R = rois.shape[0]
    batch = rois[:, 0].astype(np.int32)
    x1 = rois[:, 1].astype(np.float64) * SCALE
    y1 = rois[:, 2].astype(np.float64) * SCALE
    x2 = rois[:, 3].astype(np.float64) * SCALE
    y2 = rois[:, 4].astype(np.float64) * SCALE
    bw = (x2 - x1) / POOLED
    bh = (y2 - y1) / POOLED
    pw = np.arange(POOLED, dtype=np.float64)
    xs = x1[:, None] + pw * bw[:, None]
    ys = y1[:, None] + pw * bh[:, None]
    Ix = _tent_integral(xs, xs + bw[:, None], W)       # [R,7,W]
    Iy = _tent_integral(ys, ys + bh[:, None], H)       # [R,7,H]
    area = bw * bh
    scl = np.where(area > 0, 1.0 / np.maximum(area, 1e-12), 0.0)
    Iy_s = Iy * scl[:, None, None]

    core_groups = [[] for _ in range(NCORES)]   # (ke, kx, r, p)
    for r in range(R):
        c = int(batch[r]) % NCORES
        for p in range(POOLED):
            nz = np.nonzero(Iy_s[r, p] != 0)[0]
            if len(nz):
                ke, kx = int(nz[0]) // CHUNK_H, int(nz[-1]) // CHUNK_H
            else:
                ke, kx = 0, 0
            core_groups[c].append((ke, kx, r, p))

    ent = np.zeros((NCORES, NCHUNK), dtype=np.int64)
    for c in range(NCORES):
        for (ke, kx, r, p) in core_groups[c]:
            ent[c, ke] += 1
    quota = ent.max(axis=0)
    base = np.concatenate([[0], quota.cumsum()])
    NSLOT = int(base[-1])

    # Per-core placement: entrants of chunk k fill the bucket range
    # [base[k], base[k+1]) top-down (latest exit highest); never overflows
    # because quota[k] >= ent[c, k] for every core.
    slot_of = [dict() for _ in range(NCORES)]
    for c in range(NCORES):
        byk = {}
        for g in core_groups[c]:
            byk.setdefault(g[0], []).append(g)
        for k, gl in byk.items():
            gl.sort(key=lambda t: -t[1])
            for i, (ke, kx, r, p) in enumerate(gl):
                slot_of[c][(r, p)] = int(base[k + 1]) - 1 - i

    # alive slot interval per chunk (union over cores)
    LO = np.full(NCHUNK, 1 << 30, dtype=np.int64)
    HI = np.zeros(NCHUNK, dtype=np.int64)
    for c in range(NCORES):
        for (ke, kx, r, p) in core_groups[c]:
            s = slot_of[c][(r, p)]
            for k in range(ke, kx + 1):
                LO[k] = min(LO[k], s)
                HI[k] = max(HI[k], s + 1)
    active = HI > 0

    nblk = max(1, int(np.ceil(NSLOT / MAXSLOT_PER_BLOCK)))
    S = [round(j * NSLOT / nblk) for j in range(nblk + 1)]
    Wblk = [S[j + 1] - S[j] for j in range(nblk)]

    # pieces per chunk: (block j, s0, s1, B column offset)
    pieces = []
    offs = [0]
    for k in range(NCHUNK):
        pk = []
        if active[k]:
            for j in range(nblk):
                s0 = max(int(LO[k]), S[j])
                s1 = min(int(HI[k]), S[j + 1])
                if s1 > s0:
                    pk.append((j, s0, s1, offs[-1] + (s0 - int(LO[k])) * POOLED))
        pieces.append(pk)
        offs.append(offs[-1] + ((int(HI[k]) - int(LO[k])) * POOLED if active[k] else 0))
    NB = offs[-1]
    last_k = {}
    first_k = {}
    for k in range(NCHUNK):
        for (j, s0, s1, co) in pieces[k]:
            last_k[j] = k
            first_k.setdefault(j, k)

    # pack B [NCORES, KDIM, NB]
    B = np.zeros((NCORES, KDIM, NB), dtype=np.float32)
    IxT = Ix.transpose(0, 2, 1)    # [R, W, 7]
    for c in range(NCORES):
        for (ke, kx, r, p) in core_groups[c]:
            s = slot_of[c][(r, p)]
            for k in range(ke, kx + 1):
                col = offs[k] + (s - int(LO[k])) * POOLED
                for dh in range(CHUNK_H):
                    h = CHUNK_H * k + dh
                    v = Iy_s[r, p, h]
                    if v != 0.0:
                        B[c, dh * W:(dh + 1) * W, col:col + POOLED] = v * IxT[r]
    B = B.astype(BF16)

    # features chunk-major: FT[core, (dh,w), k*C + cc]  (bf16, no padding)
    f = features.astype(np.float32)
    ft = f.reshape(N, C, NCHUNK, CHUNK_H, W).transpose(0, 3, 4, 2, 1)
    FT = ft.reshape(N, KDIM, NCHUNK * C).astype(BF16)

    TOT = 2 * NSLOT * POOLED
    return dict(B=B, FT=FT, pieces=pieces, offs=offs, LO=LO, HI=HI, active=active,
                S=S, nblk=nblk, Wblk=Wblk, NSLOT=NSLOT, NB=NB, TOT=TOT,
                slot_of=slot_of, core_groups=core_groups, R=R,
                last_k=last_k, first_k=first_k)


def _make_key(hp):
    return (hp['NB'], hp['NSLOT'], hp['nblk'], tuple(hp['S']),
            tuple(int(x) for x in hp['LO']), tuple(int(x) for x in hp['HI']),
            tuple(bool(a) for a in hp['active']),
            tuple(tuple(p) for pk in hp['pieces'] for p in pk),
            tuple(len(pk) for pk in hp['pieces']),
            tuple(int(o) for o in hp['offs']),
            tuple(sorted(hp['last_k'].items())),
            tuple(sorted(hp['first_k'].items())))


def _build_bass(hp):
    import concourse.bass as bass  # noqa: F401
    import concourse.tile as tile
    from concourse import bacc, mybir

    NB, nblk, S, Wblk, TOT = hp['NB'], hp['nblk'], hp['S'], hp['Wblk'], hp['TOT']
    pieces, last_k, first_k = hp['pieces'], hp['last_k'], hp['first_k']

    nc = bacc.Bacc("TRN2", target_bir_lowering=False, debug=False,
                   enable_asserts=False, num_devices=NCORES)
    bf = mybir.dt.bfloat16
    f32 = mybir.dt.float32
    ft_ap = nc.dram_tensor("ft", [KDIM, NCHUNK * C], bf, kind="ExternalInput").ap()
    b_ap = nc.dram_tensor("bb", [KDIM, NB], bf, kind="ExternalInput").ap()
    out_ap = nc.dram_tensor("out", [128, TOT], bf, kind="ExternalOutput").ap()

    offs = hp['offs']
    # input DMA splits (chunk ranges): first tiny so matmuls start early
    SPLITS = [0, 2, 8, 16, 22, NCHUNK]

    with tile.TileContext(nc) as tc:
        with (
            tc.tile_pool(name="ftp", bufs=1) as ftp,
            tc.tile_pool(name="bp", bufs=1) as bp,
            tc.tile_pool(name="zp", bufs=1) as zp,
            tc.tile_pool(name="pp", bufs=1, space="PSUM") as pp,
            tc.tile_pool(name="sp", bufs=1) as sp,
        ):
            ft_sb = ftp.tile([KDIM, NCHUNK * C], bf)
            b_sb = bp.tile([KDIM, NB], bf)
            zeros = zp.tile([KDIM, BANK], bf)
            stage = sp.tile([128, TOT], bf)

            nc.vector.memset(zeros[:, :], 0.0)

            for si in range(len(SPLITS) - 1):
                k0, k1 = SPLITS[si], SPLITS[si + 1]
                nc.sync.dma_start(ft_sb[:, k0 * C:k1 * C], ft_ap[:, k0 * C:k1 * C])
                o0, o1 = offs[k0], offs[k1]
                if o1 > o0:
                    nc.scalar.dma_start(b_sb[:, o0:o1], b_ap[:, o0:o1])

            # psum tiles; tag j%4 so blocks >=4 reuse the bank of block j-4
            ptile = {}
            for j in range(nblk):
                for m in range(2):
                    ptile[(j, m)] = pp.tile([128, BANK], f32,
                                            tag=f"pt{j % 4}_{m}",
                                            name=f"pt{j}_{m}")

            # upfront clears (blocks 0..3) double as PE clock warm-up
            def clear(j, m):
                nc.tensor.matmul(
                    ptile[(j, m)][:, :BANK],
                    lhsT=zeros[:, :128], rhs=zeros[:, :BANK],
                    start=True, stop=False)

            for j in range(min(nblk, 4)):
                for m in range(2):
                    clear(j, m)

            emitted_clear = {(j, m) for j in range(min(nblk, 4)) for m in range(2)}
            for k in range(NCHUNK):
                for m in range(2):
                    lhsT = ft_sb[:, k * C + m * 128: k * C + (m + 1) * 128]
                    for (j, s0, s1, co) in pieces[k]:
                        if (j, m) not in emitted_clear:
                            clear(j, m)
                            emitted_clear.add((j, m))
                        w = (s1 - s0) * POOLED
                        p0 = (s0 - S[j]) * POOLED
                        nc.tensor.matmul(
                            ptile[(j, m)][:, p0:p0 + w],
                            lhsT=lhsT,
                            rhs=b_sb[:, co:co + w],
                            start=False,
                            stop=(k == last_k[j]),
                        )
                # evacuate banks that just closed
                for j in range(nblk):
                    if last_k.get(j) == k:
                        wj = Wblk[j] * POOLED
                        cbase = 2 * S[j] * POOLED
                        nc.vector.tensor_copy(
                            stage[:, cbase:cbase + wj], ptile[(j, 0)][:, :wj])
                        nc.vector.tensor_copy(
                            stage[:, cbase + wj:cbase + 2 * wj], ptile[(j, 1)][:, :wj])
                        nc.sync.dma_start(
                            out_ap[:, cbase:cbase + 2 * wj],
                            stage[:, cbase:cbase + 2 * wj])

    nc.compile()
    return nc


def kernel(features, rois):
    global LAST_RESULTS
    import os
    from concourse import bass_utils

    features = np.asarray(features, dtype=np.float32)
    rois = np.asarray(rois, dtype=np.float32)
    hp = _host_prep(features, rois)

    key = _make_key(hp)
    nc = _kernel_cache.get(key)
    if nc is None:
        nc = _build_bass(hp)
        _kernel_cache[key] = nc

    in_maps = [{"ft": np.ascontiguousarray(hp["FT"][c]),
                "bb": np.ascontiguousarray(hp["B"][c])}
               for c in range(NCORES)]
    res = bass_utils.run_bass_kernel_spmd(nc, in_maps, core_ids=list(range(NCORES)),
                                          tmpdir=os.environ.get("BASS_TMPDIR"))
    LAST_RESULTS = res

    # unpack: out[cc, col] (bf16) -> final[r, c, p, q] (fp32)
    S, Wblk, nblk = hp['S'], hp['Wblk'], hp['nblk']
    final = np.zeros((hp['R'], C, POOLED, POOLED), dtype=np.float32)
    blk_of_slot = np.zeros(hp['NSLOT'], dtype=np.int64)
    for j in range(nblk):
        blk_of_slot[S[j]:S[j + 1]] = j
    for c in range(NCORES):
        out = np.asarray(res.results[c]["out"]).astype(np.float32)  # [128, TOT]
        for (ke, kx, r, p) in hp['core_groups'][c]:
            s = hp['slot_of'][c][(r, p)]
            j = int(blk_of_slot[s])
            wj = Wblk[j] * POOLED
            cb = 2 * S[j] * POOLED + (s - S[j]) * POOLED
            final[r, 0:128, p, :] = out[:, cb:cb + POOLED]
            final[r, 128:256, p, :] = out[:, cb + wj:cb + wj + POOLED]
    return final


# revision 9
# speedup vs baseline: 1.1748x; 1.1237x over previous
"""PrRoIPool2D (precise ROI pooling) Trainium2 kernel — 8-core SPMD.

Fused banded sweep with aligned slot packing:
  out[r,c,p,q] = sum_{h,w} F[b_r,c,h,w] * Iy[r,p,h] * Ix[r,q,w]
Each (r,p) "group" is alive over a ~2-3 h-chunk window.  Groups are assigned
to output SLOTS via per-entry-chunk quotas (quota[k] = max over cores of
groups entering at chunk k), which aligns every core's alive window to the
same slot interval — the SPMD union of per-chunk matmul intervals is then
tight.  Slots partition into PSUM banks (blocks); a zero-weights matmul
clears each bank's has_written before use (doubling as PE clock warm-up),
then per-chunk matmuls accumulate F-chunk x B-columns straight into the
final output columns.  Output is evacuated per bank as bf16.
"""

import numpy as np
import ml_dtypes

POOLED = 7
SCALE = 0.5
N, C, H, W = 8, 256, 56, 56
NCORES = 8
CHUNK_H = 2
NCHUNK = H // CHUNK_H          # 28
KDIM = CHUNK_H * W             # 112 device K rows (no padding)
BANK = 512                     # fp32 elements per PSUM bank
MAXSLOT_PER_BLOCK = BANK // POOLED  # 73
BF16 = ml_dtypes.bfloat16

_kernel_cache = {}
LAST_RESULTS = None            # BassKernelResults stash for test harnesses


def _tent_integral(start, end, n):
    i = np.arange(n, dtype=np.float64)
    a = np.clip(start[..., None] - i, -1.0, 1.0)
    b = np.clip(end[..., None] - i, -1.0, 1.0)

    def G(t):
        return np.where(t <= 0.0, 0.5 * (t + 1.0) ** 2, 1.0 - 0.5 * (1.0 - t) ** 2)

    return G(b) - G(a)


def _host_prep(features, rois):
    R = rois.shape[0]
    batch = rois[:, 0].astype(np.int32)
    x1 = rois[:, 1].astype(np.float64) * SCALE
    y1 = rois[:, 2].astype(np.float64) * SCALE
    x2 = rois[:, 3].astype(np.float64) * SCALE
    y2 = rois[:, 4].astype(np.float64) * SCALE
    bw = (x2 - x1) / POOLED
    bh = (y2 - y1) / POOLED
    pw = np.arange(POOLED, dtype=np.float64)
    xs = x1[:, None] + pw * bw[:, None]
    ys = y1[:, None] + pw * bh[:, None]
    Ix = _tent_integral(xs, xs + bw[:, None], W)       # [R,7,W]
    Iy = _tent_integral(ys, ys + bh[:, None], H)       # [R,7,H]
    area = bw * bh
    scl = np.where(area > 0, 1.0 / np.maximum(area, 1e-12), 0.0)
    Iy_s = Iy * scl[:, None, None]

    core_groups = [[] for _ in range(NCORES)]   # (ke, kx, r, p)
    for r in range(R):
        c = int(batch[r]) % NCORES
        for p in range(POOLED):
            nz = np.nonzero(Iy_s[r, p] != 0)[0]
            if len(nz):
                ke, kx = int(nz[0]) // CHUNK_H, int(nz[-1]) // CHUNK_H
            else:
                ke, kx = 0, 0
            core_groups[c].append((ke, kx, r, p))

    ent = np.zeros((NCORES, NCHUNK), dtype=np.int64)
    for c in range(NCORES):
        for (ke, kx, r, p) in core_groups[c]:
            ent[c, ke] += 1
    quota = ent.max(axis=0)
    base = np.concatenate([[0], quota.cumsum()])
    NSLOT = int(base[-1])

    # Per-core placement: entrants of chunk k fill the bucket range
    # [base[k], base[k+1]) top-down (latest exit highest); never overflows
    # because quota[k] >= ent[c, k] for every core.
    slot_of = [dict() for _ in range(NCORES)]
    for c in range(NCORES):
        byk = {}
        for g in core_groups[c]:
            byk.setdefault(g[0], []).append(g)
        for k, gl in byk.items():
            gl.sort(key=lambda t: -t[1])
            for i, (ke, kx, r, p) in enumerate(gl):
                slot_of[c][(r, p)] = int(base[k + 1]) - 1 - i

    # alive slot interval per chunk (union over cores)
    LO = np.full(NCHUNK, 1 << 30, dtype=np.int64)
    HI = np.zeros(NCHUNK, dtype=np.int64)
    for c in range(NCORES):
        for (ke, kx, r, p) in core_groups[c]:
            s = slot_of[c][(r, p)]
            for k in range(ke, kx + 1):
                LO[k] = min(LO[k], s)
                HI[k] = max(HI[k], s + 1)
    active = HI > 0

    nblk = max(1, int(np.ceil(NSLOT / MAXSLOT_PER_BLOCK)))
    S = [round(j * NSLOT / nblk) for j in range(nblk + 1)]
    Wblk = [S[j + 1] - S[j] for j in range(nblk)]

    # pieces per chunk: (block j, s0, s1, B column offset)
    pieces = []
    offs = [0]
    for k in range(NCHUNK):
        pk = []
        if active[k]:
            for j in range(nblk):
                s0 = max(int(LO[k]), S[j])
                s1 = min(int(HI[k]), S[j + 1])
                if s1 > s0:
                    pk.append((j, s0, s1, offs[-1] + (s0 - int(LO[k])) * POOLED))
        pieces.append(pk)
        offs.append(offs[-1] + ((int(HI[k]) - int(LO[k])) * POOLED if active[k] else 0))
    NB = offs[-1]
    last_k = {}
    first_k = {}
    for k in range(NCHUNK):
        for (j, s0, s1, co) in pieces[k]:
            last_k[j] = k
            first_k.setdefault(j, k)

    # pack B [NCORES, KDIM, NB]
    B = np.zeros((NCORES, KDIM, NB), dtype=np.float32)
    IxT = Ix.transpose(0, 2, 1)    # [R, W, 7]
    for c in range(NCORES):
        for (ke, kx, r, p) in core_groups[c]:
            s = slot_of[c][(r, p)]
            for k in range(ke, kx + 1):
                col = offs[k] + (s - int(LO[k])) * POOLED
                for dh in range(CHUNK_H):
                    h = CHUNK_H * k + dh
                    v = Iy_s[r, p, h]
                    if v != 0.0:
                        B[c, dh * W:(dh + 1) * W, col:col + POOLED] = v * IxT[r]
    B = B.astype(BF16)

    # features chunk-major: FT[core, (dh,w), k*C + cc]  (bf16, no padding)
    f = features.astype(np.float32)
    ft = f.reshape(N, C, NCHUNK, CHUNK_H, W).transpose(0, 3, 4, 2, 1)
    FT = ft.reshape(N, KDIM, NCHUNK * C).astype(BF16)

    TOT = 2 * NSLOT * POOLED
    return dict(B=B, FT=FT, pieces=pieces, offs=offs, LO=LO, HI=HI, active=active,
                S=S, nblk=nblk, Wblk=Wblk, NSLOT=NSLOT, NB=NB, TOT=TOT,
                slot_of=slot_of, core_groups=core_groups, R=R,
                last_k=last_k, first_k=first_k)


def _make_key(hp):
    return (hp['NB'], hp['NSLOT'], hp['nblk'], tuple(hp['S']),
            tuple(int(x) for x in hp['LO']), tuple(int(x) for x in hp['HI']),
            tuple(bool(a) for a in hp['active']),
            tuple(tuple(p) for pk in hp['pieces'] for p in pk),
            tuple(len(pk) for pk in hp['pieces']),
            tuple(int(o) for o in hp['offs']),
            tuple(sorted(hp['last_k'].items())),
            tuple(sorted(hp['first_k'].items())))


def _build_bass(hp):
    import concourse.bass as bass  # noqa: F401
    import concourse.tile as tile
    from concourse import bacc, mybir

    NB, nblk, S, Wblk, TOT = hp['NB'], hp['nblk'], hp['S'], hp['Wblk'], hp['TOT']
    pieces, last_k, first_k = hp['pieces'], hp['last_k'], hp['first_k']

    nc = bacc.Bacc("TRN2", target_bir_lowering=False, debug=False,
                   enable_asserts=False, num_devices=NCORES)
    bf = mybir.dt.bfloat16
    f32 = mybir.dt.float32
    ft_ap = nc.dram_tensor("ft", [KDIM, NCHUNK * C], bf, kind="ExternalInput").ap()
    b_ap = nc.dram_tensor("bb", [KDIM, NB], bf, kind="ExternalInput").ap()
    out_ap = nc.dram_tensor("out", [128, TOT], bf, kind="ExternalOutput").ap()

    offs = hp['offs']
    # input DMA splits (chunk ranges): first small so matmuls start early
    SPLITS = [0, 4, 16, NCHUNK]

    with tile.TileContext(nc) as tc:
        with (
            tc.tile_pool(name="ftp", bufs=1) as ftp,
            tc.tile_pool(name="bp", bufs=1) as bp,
            tc.tile_pool(name="zp", bufs=1) as zp,
            tc.tile_pool(name="pp", bufs=1, space="PSUM") as pp,
            tc.tile_pool(name="sp", bufs=1) as sp,
        ):
            ft_sb = ftp.tile([KDIM, NCHUNK * C], bf)
            b_sb = bp.tile([KDIM, NB], bf)
            zeros = zp.tile([1, BANK], bf)
            stage = sp.tile([128, TOT], bf)

            nc.vector.memset(zeros[:, :], 0.0)

            for si in range(len(SPLITS) - 1):
                k0, k1 = SPLITS[si], SPLITS[si + 1]
                nc.sync.dma_start(ft_sb[:, k0 * C:k1 * C], ft_ap[:, k0 * C:k1 * C])
                o0, o1 = offs[k0], offs[k1]
                if o1 > o0:
                    nc.scalar.dma_start(b_sb[:, o0:o1], b_ap[:, o0:o1])

            # psum tiles; tag j%4 so blocks >=4 reuse the bank of block j-4
            ptile = {}
            for j in range(nblk):
                for m in range(2):
                    ptile[(j, m)] = pp.tile([128, BANK], f32,
                                            tag=f"pt{j % 4}_{m}",
                                            name=f"pt{j}_{m}")

            # upfront clears (blocks 0..3) double as PE clock warm-up.
            # K=1 zero-weights matmul: sets has_written on the whole bank and
            # writes 0.0, so every real matmul can run start=False.
            def clear(j, m):
                nc.tensor.matmul(
                    ptile[(j, m)][:, :BANK],
                    lhsT=zeros[0:1, :128], rhs=zeros[0:1, :BANK],
                    start=True, stop=False)

            for j in range(min(nblk, 4)):
                for m in range(2):
                    clear(j, m)

            emitted_clear = {(j, m) for j in range(min(nblk, 4)) for m in range(2)}
            for k in range(NCHUNK):
                for m in range(2):
                    lhsT = ft_sb[:, k * C + m * 128: k * C + (m + 1) * 128]
                    for (j, s0, s1, co) in pieces[k]:
                        if (j, m) not in emitted_clear:
                            clear(j, m)
                            emitted_clear.add((j, m))
                        w = (s1 - s0) * POOLED
                        p0 = (s0 - S[j]) * POOLED
                        nc.tensor.matmul(
                            ptile[(j, m)][:, p0:p0 + w],
                            lhsT=lhsT,
                            rhs=b_sb[:, co:co + w],
                            start=False,
                            stop=(k == last_k[j]),
                        )
                # evacuate banks that just closed
                for j in range(nblk):
                    if last_k.get(j) == k:
                        wj = Wblk[j] * POOLED
                        cbase = 2 * S[j] * POOLED
                        nc.vector.tensor_copy(
                            stage[:, cbase:cbase + wj], ptile[(j, 0)][:, :wj])
                        nc.vector.tensor_copy(
                            stage[:, cbase + wj:cbase + 2 * wj], ptile[(j, 1)][:, :wj])
                        nc.sync.dma_start(
                            out_ap[:, cbase:cbase + 2 * wj],
                            stage[:, cbase:cbase + 2 * wj])

    nc.compile()
    return nc


def kernel(features, rois):
    global LAST_RESULTS
    import os
    from concourse import bass_utils

    features = np.asarray(features, dtype=np.float32)
    rois = np.asarray(rois, dtype=np.float32)
    hp = _host_prep(features, rois)

    key = _make_key(hp)
    nc = _kernel_cache.get(key)
    if nc is None:
        nc = _build_bass(hp)
        _kernel_cache[key] = nc

    in_maps = [{"ft": np.ascontiguousarray(hp["FT"][c]),
                "bb": np.ascontiguousarray(hp["B"][c])}
               for c in range(NCORES)]
    res = bass_utils.run_bass_kernel_spmd(nc, in_maps, core_ids=list(range(NCORES)),
                                          tmpdir=os.environ.get("BASS_TMPDIR"))
    LAST_RESULTS = res

    # unpack: out[cc, col] (bf16) -> final[r, c, p, q] (fp32)
    S, Wblk, nblk = hp['S'], hp['Wblk'], hp['nblk']
    final = np.zeros((hp['R'], C, POOLED, POOLED), dtype=np.float32)
    blk_of_slot = np.zeros(hp['NSLOT'], dtype=np.int64)
    for j in range(nblk):
        blk_of_slot[S[j]:S[j + 1]] = j
    for c in range(NCORES):
        out = np.asarray(res.results[c]["out"]).astype(np.float32)  # [128, TOT]
        for (ke, kx, r, p) in hp['core_groups'][c]:
            s = hp['slot_of'][c][(r, p)]
            j = int(blk_of_slot[s])
            wj = Wblk[j] * POOLED
            cb = 2 * S[j] * POOLED + (s - S[j]) * POOLED
            final[r, 0:128, p, :] = out[:, cb:cb + POOLED]
            final[r, 128:256, p, :] = out[:, cb + wj:cb + wj + POOLED]
    return final


# revision 12
# speedup vs baseline: 1.2457x; 1.0603x over previous
"""PrRoIPool2D (precise ROI pooling) Trainium2 kernel — 8-core SPMD.

Fused banded sweep with aligned slot packing:
  out[r,c,p,q] = sum_{h,w} F[b_r,c,h,w] * Iy[r,p,h] * Ix[r,q,w]
Each (r,p) "group" is alive over a ~2-3 h-chunk window.  Groups are assigned
to output SLOTS via per-entry-chunk quotas (quota[k] = max over cores of
groups entering at chunk k), which aligns every core's alive window to the
same slot interval — the SPMD union of per-chunk matmul intervals is then
tight.  Slots partition into PSUM banks (blocks); a zero-weights matmul
clears each bank's has_written before use (doubling as PE clock warm-up),
then per-chunk matmuls accumulate F-chunk x B-columns straight into the
final output columns.  Output is evacuated per bank as bf16.
"""

import numpy as np
import ml_dtypes

POOLED = 7
SCALE = 0.5
N, C, H, W = 8, 256, 56, 56
NCORES = 8
CHUNK_H = 2
NCHUNK = H // CHUNK_H          # 28
KDIM = CHUNK_H * W             # 112 device K rows (no padding)
BANK = 512                     # fp32 elements per PSUM bank
MAXSLOT_PER_BLOCK = BANK // POOLED  # 73
BF16 = ml_dtypes.bfloat16

_kernel_cache = {}
LAST_RESULTS = None            # BassKernelResults stash for test harnesses


def _tent_integral(start, end, n):
    i = np.arange(n, dtype=np.float64)
    a = np.clip(start[..., None] - i, -1.0, 1.0)
    b = np.clip(end[..., None] - i, -1.0, 1.0)

    def G(t):
        return np.where(t <= 0.0, 0.5 * (t + 1.0) ** 2, 1.0 - 0.5 * (1.0 - t) ** 2)

    return G(b) - G(a)


def _host_prep(features, rois):
    R = rois.shape[0]
    batch = rois[:, 0].astype(np.int32)
    x1 = rois[:, 1].astype(np.float64) * SCALE
    y1 = rois[:, 2].astype(np.float64) * SCALE
    x2 = rois[:, 3].astype(np.float64) * SCALE
    y2 = rois[:, 4].astype(np.float64) * SCALE
    bw = (x2 - x1) / POOLED
    bh = (y2 - y1) / POOLED
    pw = np.arange(POOLED, dtype=np.float64)
    xs = x1[:, None] + pw * bw[:, None]
    ys = y1[:, None] + pw * bh[:, None]
    Ix = _tent_integral(xs, xs + bw[:, None], W)       # [R,7,W]
    Iy = _tent_integral(ys, ys + bh[:, None], H)       # [R,7,H]
    area = bw * bh
    scl = np.where(area > 0, 1.0 / np.maximum(area, 1e-12), 0.0)
    Iy_s = Iy * scl[:, None, None]

    core_groups = [[] for _ in range(NCORES)]   # (ke, kx, r, p)
    for r in range(R):
        c = int(batch[r]) % NCORES
        for p in range(POOLED):
            nz = np.nonzero(Iy_s[r, p] != 0)[0]
            if len(nz):
                ke, kx = int(nz[0]) // CHUNK_H, int(nz[-1]) // CHUNK_H
            else:
                ke, kx = 0, 0
            core_groups[c].append((ke, kx, r, p))

    ent = np.zeros((NCORES, NCHUNK), dtype=np.int64)
    for c in range(NCORES):
        for (ke, kx, r, p) in core_groups[c]:
            ent[c, ke] += 1
    quota = ent.max(axis=0)
    base = np.concatenate([[0], quota.cumsum()])
    NSLOT = int(base[-1])

    # Per-core placement: entrants of chunk k fill the bucket range
    # [base[k], base[k+1]) top-down (latest exit highest); never overflows
    # because quota[k] >= ent[c, k] for every core.
    slot_of = [dict() for _ in range(NCORES)]
    for c in range(NCORES):
        byk = {}
        for g in core_groups[c]:
            byk.setdefault(g[0], []).append(g)
        for k, gl in byk.items():
            gl.sort(key=lambda t: -t[1])
            for i, (ke, kx, r, p) in enumerate(gl):
                slot_of[c][(r, p)] = int(base[k + 1]) - 1 - i

    # alive slot interval per chunk (union over cores)
    LO = np.full(NCHUNK, 1 << 30, dtype=np.int64)
    HI = np.zeros(NCHUNK, dtype=np.int64)
    for c in range(NCORES):
        for (ke, kx, r, p) in core_groups[c]:
            s = slot_of[c][(r, p)]
            for k in range(ke, kx + 1):
                LO[k] = min(LO[k], s)
                HI[k] = max(HI[k], s + 1)
    active = HI > 0

    # block partition of slot space: full-width banks, with a small final
    # block so the tail (last cast + out DMA) is short
    S = [0]
    while NSLOT - S[-1] > MAXSLOT_PER_BLOCK:
        S.append(S[-1] + MAXSLOT_PER_BLOCK)
    rem = NSLOT - S[-1]
    if rem > 28 and len(S) > 1:
        S.append(S[-1] + (rem + 1) // 2)
    S.append(NSLOT)
    nblk = len(S) - 1
    Wblk = [S[j + 1] - S[j] for j in range(nblk)]

    # pieces per chunk: (block j, s0, s1, B column offset)
    pieces = []
    offs = [0]
    for k in range(NCHUNK):
        pk = []
        if active[k]:
            for j in range(nblk):
                s0 = max(int(LO[k]), S[j])
                s1 = min(int(HI[k]), S[j + 1])
                if s1 > s0:
                    pk.append((j, s0, s1, offs[-1] + (s0 - int(LO[k])) * POOLED))
        pieces.append(pk)
        offs.append(offs[-1] + ((int(HI[k]) - int(LO[k])) * POOLED if active[k] else 0))
    NB = offs[-1]
    last_k = {}
    first_k = {}
    for k in range(NCHUNK):
        for (j, s0, s1, co) in pieces[k]:
            last_k[j] = k
            first_k.setdefault(j, k)

    # pack B [NCORES, KDIM, NB]
    B = np.zeros((NCORES, KDIM, NB), dtype=np.float32)
    IxT = Ix.transpose(0, 2, 1)    # [R, W, 7]
    for c in range(NCORES):
        for (ke, kx, r, p) in core_groups[c]:
            s = slot_of[c][(r, p)]
            for k in range(ke, kx + 1):
                col = offs[k] + (s - int(LO[k])) * POOLED
                for dh in range(CHUNK_H):
                    h = CHUNK_H * k + dh
                    v = Iy_s[r, p, h]
                    if v != 0.0:
                        B[c, dh * W:(dh + 1) * W, col:col + POOLED] = v * IxT[r]
    B = B.astype(BF16)

    # features chunk-major: FT[core, (dh,w), k*C + cc]  (bf16, no padding)
    f = features.astype(np.float32)
    ft = f.reshape(N, C, NCHUNK, CHUNK_H, W).transpose(0, 3, 4, 2, 1)
    FT = ft.reshape(N, KDIM, NCHUNK * C).astype(BF16)

    TOT = 2 * NSLOT * POOLED
    return dict(B=B, FT=FT, pieces=pieces, offs=offs, LO=LO, HI=HI, active=active,
                S=S, nblk=nblk, Wblk=Wblk, NSLOT=NSLOT, NB=NB, TOT=TOT,
                slot_of=slot_of, core_groups=core_groups, R=R,
                last_k=last_k, first_k=first_k)


def _make_key(hp):
    return (hp['NB'], hp['NSLOT'], hp['nblk'], tuple(hp['S']),
            tuple(int(x) for x in hp['LO']), tuple(int(x) for x in hp['HI']),
            tuple(bool(a) for a in hp['active']),
            tuple(tuple(p) for pk in hp['pieces'] for p in pk),
            tuple(len(pk) for pk in hp['pieces']),
            tuple(int(o) for o in hp['offs']),
            tuple(sorted(hp['last_k'].items())),
            tuple(sorted(hp['first_k'].items())))


def _build_bass(hp):
    import concourse.bass as bass  # noqa: F401
    import concourse.tile as tile
    from concourse import bacc, mybir

    NB, nblk, S, Wblk, TOT = hp['NB'], hp['nblk'], hp['S'], hp['Wblk'], hp['TOT']
    pieces, last_k, first_k = hp['pieces'], hp['last_k'], hp['first_k']

    nc = bacc.Bacc("TRN2", target_bir_lowering=False, debug=False,
                   enable_asserts=False, num_devices=NCORES)
    bf = mybir.dt.bfloat16
    f32 = mybir.dt.float32
    ft_ap = nc.dram_tensor("ft", [KDIM, NCHUNK * C], bf, kind="ExternalInput").ap()
    b_ap = nc.dram_tensor("bb", [KDIM, NB], bf, kind="ExternalInput").ap()
    out_ap = nc.dram_tensor("out", [128, TOT], bf, kind="ExternalOutput").ap()

    offs = hp['offs']
    # input DMA splits (chunk ranges): fine-grained so the per-split
    # completion semaphores release matmuls as the data streams in
    SPLITS = list(range(0, NCHUNK + 1, 4))

    with tile.TileContext(nc) as tc:
        with (
            tc.tile_pool(name="ftp", bufs=1) as ftp,
            tc.tile_pool(name="bp", bufs=1) as bp,
            tc.tile_pool(name="zp", bufs=1) as zp,
            tc.tile_pool(name="pp", bufs=1, space="PSUM") as pp,
            tc.tile_pool(name="sp", bufs=1) as sp,
        ):
            ft_sb = ftp.tile([KDIM, NCHUNK * C], bf)
            b_sb = bp.tile([KDIM, NB], bf)
            zeros = zp.tile([1, BANK], bf)
            stage = sp.tile([128, TOT], bf)

            nc.vector.memset(zeros[:, :], 0.0)

            for si in range(len(SPLITS) - 1):
                k0, k1 = SPLITS[si], SPLITS[si + 1]
                nc.sync.dma_start(ft_sb[:, k0 * C:k1 * C], ft_ap[:, k0 * C:k1 * C])
                o0, o1 = offs[k0], offs[k1]
                if o1 > o0:
                    nc.scalar.dma_start(b_sb[:, o0:o1], b_ap[:, o0:o1])

            # psum tiles; tag j%4 so blocks >=4 reuse the bank of block j-4
            ptile = {}
            for j in range(nblk):
                for m in range(2):
                    ptile[(j, m)] = pp.tile([128, BANK], f32,
                                            tag=f"pt{j % 4}_{m}",
                                            name=f"pt{j}_{m}")

            # upfront clears (blocks 0..3) double as PE clock warm-up.
            # K=1 zero-weights matmul: sets has_written on the whole bank and
            # writes 0.0, so every real matmul can run start=False.
            def clear(j, m):
                nc.tensor.matmul(
                    ptile[(j, m)][:, :BANK],
                    lhsT=zeros[0:1, :128], rhs=zeros[0:1, :BANK],
                    start=True, stop=False)

            for j in range(min(nblk, 2)):
                for m in range(2):
                    clear(j, m)

            emitted_clear = {(j, m) for j in range(min(nblk, 2)) for m in range(2)}
            for k in range(NCHUNK):
                for m in range(2):
                    lhsT = ft_sb[:, k * C + m * 128: k * C + (m + 1) * 128]
                    for (j, s0, s1, co) in pieces[k]:
                        if (j, m) not in emitted_clear:
                            clear(j, m)
                            emitted_clear.add((j, m))
                        w = (s1 - s0) * POOLED
                        p0 = (s0 - S[j]) * POOLED
                        nc.tensor.matmul(
                            ptile[(j, m)][:, p0:p0 + w],
                            lhsT=lhsT,
                            rhs=b_sb[:, co:co + w],
                            start=False,
                            stop=(k == last_k[j]),
                        )
                # evacuate banks that just closed
                for j in range(nblk):
                    if last_k.get(j) == k:
                        wj = Wblk[j] * POOLED
                        cbase = 2 * S[j] * POOLED
                        nc.vector.tensor_copy(
                            stage[:, cbase:cbase + wj], ptile[(j, 0)][:, :wj])
                        nc.vector.tensor_copy(
                            stage[:, cbase + wj:cbase + 2 * wj], ptile[(j, 1)][:, :wj])
                        nc.sync.dma_start(
                            out_ap[:, cbase:cbase + 2 * wj],
                            stage[:, cbase:cbase + 2 * wj])

    nc.compile()
    return nc


def kernel(features, rois):
    global LAST_RESULTS
    import os
    from concourse import bass_utils

    features = np.asarray(features, dtype=np.float32)
    rois = np.asarray(rois, dtype=np.float32)
    hp = _host_prep(features, rois)

    key = _make_key(hp)
    nc = _kernel_cache.get(key)
    if nc is None:
        nc = _build_bass(hp)
        _kernel_cache[key] = nc

    in_maps = [{"ft": np.ascontiguousarray(hp["FT"][c]),
                "bb": np.ascontiguousarray(hp["B"][c])}
               for c in range(NCORES)]
    res = bass_utils.run_bass_kernel_spmd(nc, in_maps, core_ids=list(range(NCORES)),
                                          tmpdir=os.environ.get("BASS_TMPDIR"))
    LAST_RESULTS = res

    # unpack: out[cc, col] (bf16) -> final[r, c, p, q] (fp32)
    S, Wblk, nblk = hp['S'], hp['Wblk'], hp['nblk']
    final = np.zeros((hp['R'], C, POOLED, POOLED), dtype=np.float32)
    blk_of_slot = np.zeros(hp['NSLOT'], dtype=np.int64)
    for j in range(nblk):
        blk_of_slot[S[j]:S[j + 1]] = j
    for c in range(NCORES):
        out = np.asarray(res.results[c]["out"]).astype(np.float32)  # [128, TOT]
        for (ke, kx, r, p) in hp['core_groups'][c]:
            s = hp['slot_of'][c][(r, p)]
            j = int(blk_of_slot[s])
            wj = Wblk[j] * POOLED
            cb = 2 * S[j] * POOLED + (s - S[j]) * POOLED
            final[r, 0:128, p, :] = out[:, cb:cb + POOLED]
            final[r, 128:256, p, :] = out[:, cb + wj:cb + wj + POOLED]
    return final
